# revision 14
# baseline (speedup 1.0000x reference)
"""Trainium2 Bass kernel for a decoder block (MHA + GELU MLP, pre-LN, causal).

Problem shapes (hardcoded): B=2, T=2048, C=512, H=8, HD=64, f32 in/out.

Sharding: 8 cores = 2 batches x 4 query-slot groups. Causal work is balanced
across the 4 cores of a batch by giving each core four 128-token query tiles
with program-uniform key spans (16, 12, 8, 4) key-tiles:
  core j owns original query tiles {15-j, 11-j, 7-j, 3-j}  (slot order).
Keys stay in ABSOLUTE order (no rotation); at attention round r (= absolute
key tile r) only the slots whose span exceeds r are active, so the score/
exp/PV width shrinks 512->384->256->128 over rounds. Per-core causality
(diagonal triangle + rounds beyond a slot's true span) is applied as an
additive -1e30 mask ON THE TENSOR ENGINE: one extra accumulating matmul per
round (lhsT = per-core mask matrix, rhs = duplicated identity), so softmax
probabilities of dead entries are exactly zero and no vector-engine mask
multiply is needed.

All matmuls run in bf16 (f32 PSUM accumulation). Engine balance:
  - PE: transposes (through the shared PSUM ring), QKV, scores (+row-tiled
    concurrent head halves), mask add, PV, denominator broadcast, proj, FFN.
  - ACT (scalar): LN rsqrt (all issued BEFORE the exp stream so the act
    table never thrashes mid-attention), softmax exp, half the transpose
    evictions, LN2 normalize, gelu.
  - DVE (vector): LN stats, Q/K/V evictions, the other transpose evictions,
    denominator reciprocal, residual adds.
  - GPSIMD: LN1 normalize (SBUF->SBUF tensor_scalar).
DMA issue is split between the sync and scalar queues to double the ramp.
"""

import os
import sys

for _p in ("/opt/trn_rl_repo",):
    if _p not in sys.path and os.path.isdir(_p):
        sys.path.insert(0, _p)

import ml_dtypes
import numpy as np

import concourse.bacc as bacc
import concourse.bass as bass
import concourse.tile as tile
from concourse import mybir
from concourse.bass_utils import run_bass_kernel_spmd

F32 = mybir.dt.float32
BF16 = mybir.dt.bfloat16
AF = mybir.ActivationFunctionType

B, T, C, H, HD = 2, 2048, 512, 8, 64
NCORES = 8
QB = 512          # query tokens per core (4 slots x 128)
NT = T // 128     # 16 key tiles
NQ = QB // 128    # 4 query slots per core
NEG = -1.0e30
SPAN = (16, 12, 8, 4)                       # program slot spans (key tiles)
WR = [128 * (4 - r // 4) for r in range(NT)]  # active query width per round

last_run = None       # test harness reads exec_time_ns from here
_prog_cache = {}


def _build_program():
    nc = bacc.Bacc("TRN2", target_bir_lowering=False, debug=False,
                   num_devices=NCORES)

    xb_d = nc.dram_tensor("xb", [T, C], BF16, kind="ExternalInput")
    xq_d = nc.dram_tensor("xqs", [QB, C], BF16, kind="ExternalInput")
    wq_d = nc.dram_tensor("wq", [128, 4, 512], BF16, kind="ExternalInput")
    wk_d = nc.dram_tensor("wk", [128, 4, 512], BF16, kind="ExternalInput")
    wv_d = nc.dram_tensor("wv", [128, 4, 512], BF16, kind="ExternalInput")
    wo_d = nc.dram_tensor("wo", [128, 4, 512], BF16, kind="ExternalInput")
    w1_d = nc.dram_tensor("w1", [128, 16, 512], BF16, kind="ExternalInput")
    w2_d = nc.dram_tensor("w2", [128, 16, 512], BF16, kind="ExternalInput")
    bo_d = nc.dram_tensor("bo", [1, 512], BF16, kind="ExternalInput")
    b1_d = nc.dram_tensor("b1c", [128, 16], F32, kind="ExternalInput")
    b2_d = nc.dram_tensor("b2r", [1, 512], BF16, kind="ExternalInput")
    id_d = nc.dram_tensor("identc", [128, 128], F32, kind="ExternalInput")
    mk_d = nc.dram_tensor("maskc", [128, 16, 4, 128], BF16, kind="ExternalInput")
    on_d = nc.dram_tensor("onesc", [128, 512], BF16, kind="ExternalInput")
    out_d = nc.dram_tensor("out", [QB, C], BF16, kind="ExternalOutput")

    with tile.TileContext(nc) as tc:
        with (
            tc.tile_pool(name="const", bufs=1) as const,
            tc.tile_pool(name="mid", bufs=1) as mid,
            tc.tile_pool(name="tp", bufs=3) as tp,
            tc.tile_pool(name="sp", bufs=4) as sp,
        ):
            # ---------------- DMAs, split across sync + scalar queues ----
            ident = const.tile([128, 128], F32)
            nc.sync.dma_start(ident[:], id_d[:])
            xqs_sb = mid.tile([128, 4, 512], BF16)   # selected q tiles
            x_sb = mid.tile([128, 16, 512], BF16)    # absolute-order x
            for qt in range(NQ):
                nc.sync.dma_start(xqs_sb[:, qt, :], xq_d[bass.ts(qt, 128), :])
            wq_sb = const.tile([128, 4, 512], BF16)
            nc.scalar.dma_start(wq_sb[:], wq_d[:])
            for t in range(0, 4):
                nc.scalar.dma_start(x_sb[:, t, :], xb_d[bass.ts(t, 128), :])
            wk_sb = const.tile([128, 4, 512], BF16)
            nc.sync.dma_start(wk_sb[:], wk_d[:])
            wv_sb = const.tile([128, 4, 512], BF16)
            nc.scalar.dma_start(wv_sb[:], wv_d[:])
            mask_sb = const.tile([128, 16, 4, 128], BF16)
            nc.sync.dma_start(mask_sb[:], mk_d[:])
            ones512 = const.tile([128, 512], BF16)
            nc.sync.dma_start(ones512[:], on_d[:])
            ones_sb = ones512
            for t in range(4, 16):
                nc.sync.dma_start(x_sb[:, t, :], xb_d[bass.ts(t, 128), :])
            wo_sb = const.tile([128, 4, 512], BF16)
            nc.sync.dma_start(wo_sb[:], wo_d[:])
            bo_sb = const.tile([1, 512], BF16)
            nc.sync.dma_start(bo_sb[:], bo_d[:])
            b1_sb = const.tile([128, 16], F32)
            nc.sync.dma_start(b1_sb[:], b1_d[:])
            b2_sb = const.tile([1, 512], BF16)
            nc.sync.dma_start(b2_sb[:], b2_d[:])
            w1_sb = const.tile([128, 16, 512], BF16)
            nc.sync.dma_start(w1_sb[:], w1_d[:])
            w2_sb = const.tile([128, 16, 512], BF16)
            nc.sync.dma_start(w2_sb[:], w2_d[:])

            eps_sb = const.tile([128, 1], F32)
            nc.vector.memset(eps_sb[:], 1e-5)

            # ---------------- persistent mid tensors ----------------
            hq_sb = mid.tile([128, 4, 512], BF16)    # ln1(xqs)^T (slot order)
            h1t_sb = mid.tile([128, 4, 2048], BF16)  # ln1(x)^T (absolute)
            kt_sb = mid.tile([128, 4, 2048], BF16)   # K^T (head pair, 64h+d)
            v_sb = mid.tile([128, 16, 520], BF16)    # V + ones column per head
            qt_sb = mid.tile([128, 4, 512], BF16)    # Q^T (slot order)
            at_sb = mid.tile([128, 4, 512], BF16)    # attnT (normalized)
            x2_sb = mid.tile([128, 4, 512], F32)     # post-attn residual
            h2t_sb = mid.tile([128, 4, 512], BF16)   # ln2(x2)^T
            # persistent LN1 stats: 20 tiles (4 xqs + 16 x)
            mv_sb = mid.tile([128, 20, 2], F32)
            rs_sb = mid.tile([128, 20], F32)

            # pre-set the ones columns of V (col 64 of each 65-wide group)
            vones = (v_sb[:, :, :]
                     .rearrange("p a (h e) -> p a h e", e=65)[:, :, :, 64:65])
            nc.vector.tensor_copy(
                vones, ones512[:, 0:128]
                .rearrange("p (a h) -> p a h", h=8).unsqueeze(3))

            def ln_stats(idx, src_ap):
                st = sp.tile([128, 6], F32, tag="st")
                nc.vector.bn_stats(out=st[:], in_=src_ap)
                nc.vector.bn_aggr(out=mv_sb[:, idx, :], in_=st[:])
                # sigma on ACT (all issued pre-exp), reciprocal on DVE
                lg = sp.tile([128, 1], F32, tag="lg")
                nc.scalar.activation(out=lg[:], in_=mv_sb[:, idx, 1:2],
                                     func=AF.Sqrt, bias=eps_sb[:])
                nc.vector.reciprocal(out=rs_sb[:, idx:idx + 1], in_=lg[:])

            def ln_norm(idx, src_ap, dst_ap, eng):
                eng.tensor_scalar(
                    out=dst_ap, in0=src_ap, scalar1=mv_sb[:, idx, 0:1],
                    scalar2=rs_sb[:, idx:idx + 1],
                    op0=mybir.AluOpType.subtract, op1=mybir.AluOpType.mult)

            # ==== attention: shared 2-deep PSUM ring + po accumulators ====
            with (
                tc.tile_pool(name="ps2", bufs=2, space="PSUM") as ps2_ps,
                tc.tile_pool(name="ap", bufs=6) as ap_pool,
            ):
                def ring():
                    return ps2_ps.tile([128, 2, 512], F32, tag="ps",
                                       name="ring")

                def ln_finish(idx, src_ap, dst3, col, norm_eng, ev_eng):
                    # norm -> transpose (f32, through the shared ring) -> evict
                    ht = tp.tile([128, 512], F32, tag="ht")
                    ln_norm(idx, src_ap, ht[:], norm_eng)
                    rt = ring()
                    pst = rt.rearrange("p a (b c) -> p (a b) c",
                                       c=128)[:, 0:4, :]
                    for cc in range(4):
                        nc.tensor.transpose(
                            pst[:, cc, :], ht[:, bass.ts(cc, 128)], ident[:])
                    ev = dst3[:, :, bass.ts(col, 128)]
                    if ev_eng is nc.scalar:
                        nc.scalar.copy(ev, pst[:])
                    else:
                        ev_eng.tensor_copy(ev, pst[:])

                def lnq_tile(qt, norm_eng=nc.vector, ev_eng=nc.scalar):
                    ln_finish(qt, xqs_sb[:, qt, :], hq_sb, qt,
                              norm_eng, ev_eng)

                def ln_tile(t, norm_eng=nc.vector, ev_eng=nc.scalar):
                    ln_finish(4 + t, x_sb[:, t, :], h1t_sb, t,
                              norm_eng, ev_eng)

                def qt_pair(i):
                    rt = ring()
                    for j in range(2):
                        pr = 2 * i + j
                        for cc in range(4):
                            nc.tensor.matmul(
                                rt[:, j, :], wq_sb[:, cc, bass.ts(pr, 128)],
                                hq_sb[:, cc, :],
                                start=(cc == 0), stop=(cc == 3))
                    nc.vector.tensor_copy(qt_sb[:, 2 * i:2 * i + 2, :], rt[:])

                def kt_pair(nk, i):
                    rt = ring()
                    for j in range(2):
                        pr = 2 * i + j
                        for cc in range(4):
                            nc.tensor.matmul(
                                rt[:, j, :], wk_sb[:, cc, bass.ts(pr, 128)],
                                h1t_sb[:, cc, bass.ts(nk, 512)],
                                start=(cc == 0), stop=(cc == 3))
                    ev = kt_sb[:, 2 * i:2 * i + 2, bass.ts(nk, 512)]
                    nc.vector.tensor_copy(ev, rt[:])

                def v_pair(t):
                    rt = ring()
                    for j in range(2):
                        for cc in range(4):
                            nc.tensor.matmul(
                                rt[:, j, :],
                                h1t_sb[:, cc, bass.ts(t + j, 128)],
                                wv_sb[:, cc, :],
                                start=(cc == 0), stop=(cc == 3))
                    ev = (v_sb[:, t:t + 2, :]
                          .rearrange("p a (h e) -> p a h e", e=65)[:, :, :, 0:64])
                    sv = rt[:].rearrange("p a (h e) -> p a h e", e=64)
                    nc.vector.tensor_copy(ev, sv)

                # ---- lead-in: stats+sqrt for ALL tiles pre-exp; LN
                # finish only for what round 0 needs ----
                for qt in range(NQ):
                    ln_stats(qt, xqs_sb[:, qt, :])
                for qt in range(NQ):
                    lnq_tile(qt, ev_eng=(nc.scalar if qt % 2 else
                                         nc.vector))
                qt_pair(0)
                qt_pair(1)
                for t in range(0, 4):
                    ln_stats(4 + t, x_sb[:, t, :])
                for t in range(0, 4):
                    ln_tile(t, ev_eng=(nc.scalar if t % 2 else nc.vector))
                kt_pair(0, 0)
                kt_pair(0, 1)
                v_pair(0)
                for t in range(4, 16):
                    ln_stats(4 + t, x_sb[:, t, :])

                with tc.tile_pool(name="psO", bufs=1, space="PSUM") as po_ps:
                    LAG = 2
                    po = {}
                    pts = {}

                    def alloc_po(pr):
                        # 4 tags (2 passes x 2 halves), 1 buf each: pass B
                        # reuses pass A's banks once its tails have drained
                        po[pr] = [po_ps.tile([65, 512], F32,
                                             tag=f"po{pr % 2}_{i}",
                                             name=f"po{pr}_{i}", bufs=1)
                                  for i in range(2)]

                    def pv_pair(pr, r):
                        pt_s, sub = pts.pop((pr, r))
                        w = WR[r]
                        for half in range(2):
                            h = 2 * pr + half
                            nc.tensor.matmul(
                                po[pr][half][:, 0:w],
                                v_sb[:, r, h * 65:(h + 1) * 65],
                                pt_s[:, 2 * sub + half, 0:w],
                                start=(r == 0), stop=(r == NT - 1),
                                skip_group_check=True)

                    def score_exp(pr, r, ptile, sub):
                        # sub = 0/1: which half of the shared [128,4,512]
                        # ptile this head-pair writes
                        w = WR[r]
                        pss = ring()
                        for half in range(2):
                            base = 64 * half
                            nc.tensor.matmul(
                                pss[:, half, 0:w],
                                kt_sb[base:base + 64, pr, bass.ts(r, 128)],
                                qt_sb[base:base + 64, pr, 0:w],
                                start=True, stop=True)
                        nc.scalar.activation(
                            out=ptile[:, 2 * sub:2 * sub + 2, 0:w],
                            in_=pss[:, :, 0:w], func=AF.Exp)
                        pts[(pr, r)] = (ptile, sub)

                    def score_round(pra, prb, r):
                        # both head-pairs of a pass share one ptile so the
                        # causal mask lands in a single DVE multiply
                        w = WR[r]
                        ptile = ap_pool.tile([128, 4, 512], BF16, tag="pt",
                                             bufs=6, name="ptile")
                        score_exp(pra, r, ptile, 0)
                        score_exp(prb, r, ptile, 1)
                        nc.vector.tensor_mul(
                            out=ptile[:, :, w - 128:w],
                            in0=ptile[:, :, w - 128:w],
                            in1=mask_sb[:, r, :, :])

                    def attn_tail(pr):
                        for half in range(2):
                            base = 64 * half
                            dn = ap_pool.tile([1, 512], BF16, tag="dn", bufs=4)
                            with nc.allow_low_precision(
                                    reason="softmax denominator to bf16; "
                                    "~0.4% relative, inside the 2e-2 gate"):
                                nc.vector.tensor_copy(
                                    dn[:], po[pr][half][64:65, :])
                            bc = ring()
                            nc.tensor.matmul(
                                bc[0:64, 0, :], ones_sb[0:1, 0:64], dn[:],
                                start=True, stop=True)
                            rb = ap_pool.tile([64, 512], F32, tag="rb", bufs=2)
                            nc.vector.reciprocal_approx_fast(
                                out=rb[:], in_=bc[0:64, 0, :])
                            nc.vector.tensor_mul(
                                out=at_sb[base:base + 64, pr, :],
                                in0=po[pr][half][0:64, :], in1=rb[:])

                    # per-round production steps for pass A:
                    # finish LN (norm+transpose+evict) for tiles 4-15 and
                    # produce K^T/V chunks just ahead of their deadlines
                    def prod(r):
                        ev = [nc.vector, nc.scalar]
                        if r == 0:
                            ln_tile(4, ev_eng=ev[0])
                            ln_tile(5, ev_eng=ev[1])
                        elif r == 1:
                            ln_tile(6, ev_eng=ev[0])
                            ln_tile(7, ev_eng=ev[1])
                            kt_pair(1, 0)
                        elif r == 2:
                            kt_pair(1, 1)
                            v_pair(2)
                        elif r == 3:
                            v_pair(4)
                            ln_tile(8, ev_eng=ev[0])
                            ln_tile(9, ev_eng=ev[1])
                        elif r == 4:
                            ln_tile(10, ev_eng=ev[0])
                            ln_tile(11, ev_eng=ev[1])
                        elif r == 5:
                            kt_pair(2, 0)
                            v_pair(6)
                        elif r == 6:
                            kt_pair(2, 1)
                            v_pair(8)
                        elif r == 7:
                            ln_tile(12, ev_eng=ev[0])
                            ln_tile(13, ev_eng=ev[1])
                        elif r == 8:
                            ln_tile(14, ev_eng=ev[0])
                            ln_tile(15, ev_eng=ev[1])
                            v_pair(10)
                        elif r == 9:
                            kt_pair(3, 0)
                        elif r == 10:
                            kt_pair(3, 1)
                            v_pair(12)
                        elif r == 11:
                            v_pair(14)

                    alloc_po(0)
                    alloc_po(1)
                    for r in range(NT):
                        score_round(0, 1, r)
                        prod(r)
                        if r >= LAG:
                            pv_pair(0, r - LAG)
                            pv_pair(1, r - LAG)
                    for r in range(NT - LAG, NT):
                        pv_pair(0, r)
                        pv_pair(1, r)
                    # pass B pre-issue keeps the exp stream fed through tails
                    for r in range(LAG + 1):
                        score_round(2, 3, r)
                    attn_tail(0)
                    attn_tail(1)
                    alloc_po(2)
                    alloc_po(3)
                    pv2 = 0
                    for r in range(LAG + 1, NT):
                        score_round(2, 3, r)
                        for _ in range(2):
                            if pv2 <= r - LAG:
                                pv_pair(2, pv2)
                                pv_pair(3, pv2)
                                pv2 += 1
                    while pv2 < NT:
                        pv_pair(2, pv2)
                        pv_pair(3, pv2)
                        pv2 += 1
                    attn_tail(2)
                    attn_tail(3)

                # ==== output projection + residual + LN2, interleaved ====
                def ln2_tile(qt):
                    idx = sp.tile([128, 2], F32, tag="mv2")
                    st = sp.tile([128, 6], F32, tag="st")
                    nc.vector.bn_stats(out=st[:], in_=x2_sb[:, qt, :])
                    nc.vector.bn_aggr(out=idx[:], in_=st[:])
                    lg = sp.tile([128, 1], F32, tag="lg2")
                    nc.scalar.activation(out=lg[:], in_=idx[:, 1:2],
                                         func=AF.Sqrt, bias=eps_sb[:])
                    rs = sp.tile([128, 1], F32, tag="rs2")
                    nc.vector.reciprocal(out=rs[:], in_=lg[:])
                    ht = tp.tile([128, 512], F32, tag="ht")
                    nc.vector.tensor_scalar(
                        out=ht[:], in0=x2_sb[:, qt, :], scalar1=idx[:, 0:1],
                        scalar2=rs[:], op0=mybir.AluOpType.subtract,
                        op1=mybir.AluOpType.mult)
                    rt = ring()
                    pst = rt.rearrange("p a (b c) -> p (a b) c",
                                       c=128)[:, 0:4, :]
                    for cc in range(4):
                        nc.tensor.transpose(
                            pst[:, cc, :], ht[:, bass.ts(cc, 128)], ident[:])
                    nc.vector.tensor_copy(h2t_sb[:, :, bass.ts(qt, 128)],
                                          pst[:])

                with tc.tile_pool(name="pf", bufs=2, space="PSUM") as pf_ps:
                    done = []
                    for qt in range(NQ):
                        ps = pf_ps.tile([128, 512], F32, tag="pf")
                        nc.tensor.matmul(ps[:], ones_sb[0:1, 0:128],
                                         bo_sb[:], start=True, stop=False)
                        for cc in range(4):
                            nc.tensor.matmul(
                                ps[:], at_sb[:, cc, bass.ts(qt, 128)],
                                wo_sb[:, cc, :], start=False, stop=(cc == 3))
                        nc.vector.tensor_add(out=x2_sb[:, qt, :], in0=ps[:],
                                             in1=xqs_sb[:, qt, :])
                        done.append(qt)
                        if qt >= 1:
                            ln2_tile(done.pop(0))
                    for qt in done:
                        ln2_tile(qt)

            # ======== FFN1 + gelu + FFN2, fused per f-tile ========
            with (
                tc.tile_pool(name="pg", bufs=2, space="PSUM") as pg_ps,
                tc.tile_pool(name="pf2", bufs=1, space="PSUM") as pf2_ps,
                tc.tile_pool(name="gp", bufs=3) as gp,
                tc.tile_pool(name="op", bufs=2) as op,
            ):
                pso = [pf2_ps.tile([128, 512], F32, tag=f"o{qt}",
                                   name=f"o{qt}") for qt in range(NQ)]
                for qt in range(NQ):
                    nc.tensor.matmul(
                        pso[qt][:], ones_sb[0:1, 0:128], b2_sb[:],
                        start=True, stop=False, skip_group_check=True)
                gts = {}

                def ffn2(f, stop):
                    gprev = gts.pop(f)
                    for qt in range(NQ):
                        nc.tensor.matmul(
                            pso[qt][:],
                            gprev[:, bass.ts(qt, 128)],
                            w2_sb[:, f, :], start=False, stop=stop,
                            skip_group_check=True)

                for f in range(16):
                    ps = pg_ps.tile([128, 512], F32, tag="pg")
                    for cc in range(4):
                        nc.tensor.matmul(
                            ps[:], w1_sb[:, f, bass.ts(cc, 128)],
                            h2t_sb[:, cc, :],
                            start=(cc == 0), stop=(cc == 3))
                    gt = gp.tile([128, 512], BF16, tag="gt")
                    nc.scalar.activation(
                        out=gt[:], in_=ps[:], func=AF.Gelu,
                        bias=b1_sb[:, f:f + 1])
                    gts[f] = gt
                    if f >= 1:
                        ffn2(f - 1, stop=False)
                ffn2(15, stop=True)
                for qt in range(NQ):
                    ot = op.tile([128, 512], BF16, tag="ot")
                    with nc.allow_low_precision(
                            reason="bf16 output; adds ~0.2% rms inside the "
                            "2e-2 gate"):
                        nc.vector.tensor_add(out=ot[:], in0=pso[qt][:],
                                             in1=x2_sb[:, qt, :])
                    eng = nc.sync if qt % 2 == 0 else nc.scalar
                    eng.dma_start(out_d[bass.ts(qt, 128), :], ot[:])

    nc.compile()
    return nc


def _bf16(a):
    return np.ascontiguousarray(np.asarray(a, np.float32)).astype(
        ml_dtypes.bfloat16)


def _host_prep(x, Wq, Wk, Wv, Wo, bo, W1, b1, W2, b2, g1, be1, g2, be2):
    """Fold LN gains into weights; build per-core slot-gathered inputs."""
    x = np.asarray(x, np.float32)
    g1 = np.asarray(g1, np.float32)
    be1 = np.asarray(be1, np.float32)
    g2 = np.asarray(g2, np.float32)
    be2 = np.asarray(be2, np.float32)

    wq_cat = np.transpose(np.asarray(Wq, np.float32), (1, 0, 2)).reshape(C, H * HD)
    wk_cat = np.transpose(np.asarray(Wk, np.float32), (1, 0, 2)).reshape(C, H * HD)
    wv_cat = np.transpose(np.asarray(Wv, np.float32), (1, 0, 2)).reshape(C, H * HD)
    scl = float(HD) ** -0.5
    wq_f = (g1[:, None] * wq_cat) * scl
    wk_f = g1[:, None] * wk_cat
    wv_f = g1[:, None] * wv_cat
    bq = (be1 @ wq_cat) * scl
    assert not (np.any(bq) or np.any(be1 @ wk_cat) or np.any(be1 @ wv_cat)), \
        "nonzero folded QKV bias unsupported in this variant"

    W1 = np.asarray(W1, np.float32)
    w1_f = g2[:, None] * W1
    b1_f = np.asarray(b1, np.float32) + be2 @ W1

    common = {
        "identc": np.eye(128, dtype=np.float32),
        "onesc": _bf16(np.ones((128, 512), np.float32)),
        "wq": _bf16(wq_f.reshape(4, 128, 512).transpose(1, 0, 2)),
        "wk": _bf16(wk_f.reshape(4, 128, 512).transpose(1, 0, 2)),
        "wv": _bf16(wv_f.reshape(4, 128, 512).transpose(1, 0, 2)),
        "wo": _bf16(
            np.asarray(Wo, np.float32).reshape(4, 128, 512).transpose(1, 0, 2)),
        "w1": _bf16(
            np.transpose(
                w1_f.reshape(4, 128, 16, 128).transpose(2, 1, 0, 3)
                .reshape(16, 128, 512), (1, 0, 2))),
        "w2": _bf16(np.transpose(
            np.asarray(W2, np.float32).reshape(16, 128, 512), (1, 0, 2))),
        "bo": _bf16(np.asarray(bo, np.float32).reshape(1, 512)),
        "b1c": np.ascontiguousarray(b1_f.reshape(16, 128).T),
        "b2r": _bf16(np.asarray(b2, np.float32).reshape(1, 512)),
    }

    qidx = np.arange(128)
    in_maps = []
    for c in range(NCORES):
        bb, j = c // 4, c % 4
        qsel = [s - 1 - j for s in SPAN]     # original q tiles, slot order
        xqs = np.concatenate([x[bb, qt * 128:(qt + 1) * 128] for qt in qsel])
        # 0/1 keep mask, [key kk, round r, head-dup, query q]
        mk = np.zeros((128, 16, 128), np.float32)
        for r in range(16):
            s_c = 4 * (r // 4) + 4 - j       # true span of the masked slot
            if r < s_c - 1:                  # fully valid round
                mk[:, r, :] = 1.0
            elif r == s_c - 1:               # diagonal tile: lower-tri keep
                mk[:, r, :] = (qidx[:, None] <= qidx[None, :]).astype(
                    np.float32)
        im = dict(common)
        im["xb"] = _bf16(x[bb])
        im["xqs"] = _bf16(xqs)
        im["maskc"] = _bf16(np.broadcast_to(mk[:, :, None, :],
                                            (128, 16, 4, 128)))
        in_maps.append(im)
    return in_maps


def kernel(**inputs):
    global last_run
    in_maps = _host_prep(**inputs)
    if "prog" not in _prog_cache:
        _prog_cache["prog"] = _build_program()
    nc = _prog_cache["prog"]
    res = run_bass_kernel_spmd(nc, in_maps, list(range(NCORES)))
    last_run = res
    out = np.empty((B, T, C), np.float32)
    for c in range(NCORES):
        bb, j = c // 4, c % 4
        qsel = [s - 1 - j for s in SPAN]
        core_out = np.asarray(res.results[c]["out"], np.float32)
        for i, qt in enumerate(qsel):
            out[bb, qt * 128:(qt + 1) * 128, :] = \
                core_out[i * 128:(i + 1) * 128]
    return out
